# revision 40
# baseline (speedup 1.0000x reference)
"""Trainium2 Bass kernel for nn_EdgeEncoder (moe_routing).

Strategy
--------
Each of E edges is routed to 1 of 9 expert MLPs (4 -> 256 -> 256), then
  out = relu(concat([type_embed[tid], source_embed[sid], pv]) @ Wf + bf).

Host (numpy, cheap O(E) work):
  * scale/mask params, group edge indices by expert (base type),
  * split every expert's edges evenly over the 8 cores, padding each
    per-core expert segment to a multiple of 128 edges so all cores run
    ONE identical program (segment boundaries are compile-time constants),
  * algebraic fusions so the device does minimal work:
      - x gets a ones-row so b1 rides inside the layer-1 matmul,
      - V[t] = W2[t] @ Wf_pv (f64 host precompute) fuses layer 2 with the
        final projection: pv @ Wf_pv == h @ V[t] + const,
      - G_t = [type_embed @ Wf_t ; source_embed @ Wf_s ; b2@Wf_pv + bf]
        turns both embedding gathers and every bias into one K=20 matmul
        against the one-hot rows (ones-row coefficient carries the consts).

Device per 512-edge block (edges pre-grouped by expert, transposed), all
matmuls bf16 (PSUM accumulates fp32; verified rel-err ~2.7e-3 vs 2e-2 gate):
  * layer-1 (K=5) and the G matmul (K=20) are packed as concurrent
    row-group matmuls (tile_position row groups 0/32/64/96) -> they
    retire in ~one matmul slot each pair instead of four slots.
  * 4 V matmuls K=128 N=512 do the real work (2048 cycles, roofline).
  * one ACT relu [128,1024] evacuates h, one DVE relu [128,1024] emits
    the output as fp16 (halves the store traffic; host upcasts).
  * software pipeline: emit L1(b+2) after V(b)+G(b) so the ACT relu hides
    under the next block's V matmuls; PSUM = exactly 8 banks
    (2x 2-bank hts + 2x 2-bank ots).

DMA choreography (the dominant cost after the matmul restructure; this
environment moves DRAM traffic at ~25GB/s per packet-stream with ~0.7us
per dma_start on the issuing engine, so packet shape and issue placement
matter more than raw bytes):
  * per-partition contiguous runs become single DMA packets -> keep them
    4-9KB: wg as 4 band-row DMAs, xu in quarter-L chunks, stores batch
    SBATCH blocks into one [128, SBATCH*2KB-per-partition] transfer.
  * sync ring: wg bands, vr with the first-used expert's slice split out
    in front (V(0) starts ~6us earlier), then the output stores.
  * scalar ring: xu chunk 0 up-front; later chunks are issued from
    inside the block loop (between ACT relus) so their ~0.7us issue cost
    never delays relu(0..1) and never congests the steady-state ACT
    stream. A dependency-free bf16 warm burst keeps the PE busy (HAM
    clock-gate open) while the first inputs land.
"""

import math
import os

import ml_dtypes
import numpy as np

import concourse.bacc as bacc
import concourse.bass as bass
import concourse.mybir as mybir
import concourse.tile as tile
from concourse.bass_utils import run_bass_kernel_spmd

# ---- static module configuration (mirrors the torch source) ----
T = 9            # base types ("experts")
P_MAX = 4
D = 256
N_TYPES = 14
N_SRC = 5
NCORES = 8
BLOCK = 512      # edges per device block
GRP = 128        # edge group granularity (PE partition dim)
SBATCH = 4       # output blocks batched per store DMA

BASE_MAP = np.array([0, 0, 0, 1, 1, 1, 2, 2, 3, 4, 5, 6, 7, 8], dtype=np.int32)
PCOUNT = np.array([2, 2, 1, 1, 1, 1, 3, 2, 4], dtype=np.int32)
SCALES = np.ones((T, P_MAX), dtype=np.float32)
SCALES[0, :2] = [1.0, 1e-06]      # nmos  m, w
SCALES[1, :2] = [1.0, 1e-06]      # pmos  m, w
SCALES[2, 0] = 1.0                # balun rout
SCALES[3, 0] = 1000.0             # resistor r
SCALES[4, 0] = 1e-12              # capacitor c
SCALES[5, 0] = 1e-09              # inductor l
SCALES[6, :3] = [1.0, 1.0, 1.0]   # vsource dc, mag, phase
SCALES[7, :2] = [0.001, 0.001]    # isource dc, mag
SCALES[8, :4] = [1.0, 1.0, 1e9, 1.0]  # port dbm, dc, freq, num

KX = 5                            # x rows: xT(4) + ones
KU = N_TYPES + N_SRC + 1          # 20 rows: type/source one-hot + ones

_F32 = mybir.dt.float32
_BF16 = mybir.dt.bfloat16
_FP16 = mybir.dt.float16
_WARM_BURST = int(os.environ.get("EDGEENC_WARM_BURST", "24"))
_XU_CHUNKS = int(os.environ.get("EDGEENC_XU_CHUNKS", "4"))

_PROGRAM_CACHE: dict = {}
LAST_RESULT = None  # BassKernelResults of the most recent run (for test harness)


def _layout(base_ids: np.ndarray):
    """Per-expert per-core segment sizes (multiples of GRP), identical on
    every core so one program serves all 8."""
    n_t = np.bincount(base_ids, minlength=T)
    m_t = np.zeros(T, dtype=np.int64)
    for t in range(T):
        if n_t[t] > 0:
            per_core = math.ceil(n_t[t] / NCORES)
            m_t[t] = math.ceil(per_core / GRP) * GRP
    L0 = int(m_t.sum())
    L = math.ceil(L0 / BLOCK) * BLOCK
    # fold the tail pad into the last present expert's segment
    last = int(np.nonzero(m_t)[0][-1])
    m_t[last] += L - L0
    return n_t, m_t, L


def _group_experts(m_t: np.ndarray) -> np.ndarray:
    """expert id of each 128-edge group, concatenated per expert."""
    return np.repeat(np.arange(T), (m_t // GRP))


def _build_order(base_ids: np.ndarray, n_t, m_t, L) -> np.ndarray:
    """ORD[c, j] = global edge index at per-core slot j (or -1 = pad)."""
    ORD = np.full((NCORES, L), -1, dtype=np.int64)
    off = 0
    for t in range(T):
        if m_t[t] == 0:
            continue
        seg = int(m_t[t])
        idx = np.nonzero(base_ids == t)[0]
        arr = np.full(NCORES * seg, -1, dtype=np.int64)
        arr[: idx.shape[0]] = idx
        ORD[:, off : off + seg] = arr.reshape(NCORES, seg)
        off += seg
    return ORD


def _host_inputs(type_ids, source_ids, params, ORD):
    """XU[c] = [50, L] bf16: x rows (twice, for row groups 0/32) then
    u one-hot rows (twice, for row groups 64/96)."""
    base_ids = BASE_MAP[type_ids]
    scales = SCALES[base_ids]                                  # [E,4]
    validp = np.arange(P_MAX)[None, :] < PCOUNT[base_ids][:, None]
    x = np.where(validp, params.astype(np.float32) / scales, 0.0).astype(np.float32)

    L = ORD.shape[1]
    INX = np.zeros((NCORES, KX, L), dtype=np.float32)
    INU = np.zeros((NCORES, KU, L), dtype=np.float32)
    valid = ORD >= 0
    ids = ORD[valid]
    tmp = np.zeros((NCORES, L, P_MAX), dtype=np.float32)
    tmp[valid] = x[ids]
    INX[:, 0:P_MAX, :] = tmp.transpose(0, 2, 1)
    INX[:, P_MAX, :] = valid
    ci, co = np.nonzero(valid)
    INU[ci, type_ids[ids], co] = 1.0
    INU[ci, N_TYPES + source_ids[ids], co] = 1.0
    INU[:, KU - 1, :] = valid
    XU = np.concatenate([INX, INX, INU, INU], axis=1)          # [8, 50, L]
    return np.ascontiguousarray(XU.astype(ml_dtypes.bfloat16))


def _host_weights(type_embed, source_embed, W1, b1, W2, b2, Wf, bf):
    f = np.float32
    W1 = W1.astype(f); b1 = b1.astype(f); W2 = W2.astype(np.float64)
    b2 = b2.astype(f); Wf = Wf.astype(f); bf = bf.astype(f)
    type_embed = type_embed.astype(f); source_embed = source_embed.astype(f)

    W1e = np.concatenate([W1, b1[:, None, :]], axis=1)          # [9,5,256]
    Wft, Wfs, Wfp = Wf[:D], Wf[D : 2 * D], Wf[2 * D :]

    # V[t] = W2[t] @ Wf_pv (f64), fusing layer 2 with the final projection.
    # lhsT blocks: [128, 18*256]; block (t,h) = V[t][h*128:(h+1)*128,:]
    V = (W2 @ Wfp.astype(np.float64)).astype(f)                 # [9,256,256]
    VR = np.ascontiguousarray(
        V.reshape(T, 2, 128, D).transpose(2, 0, 1, 3).reshape(128, T * 2 * D)
    )

    # G_t [20,256]: type rows, source rows, const row (b2@Wf_pv + bf)
    gt = type_embed @ Wft                                       # [14,256]
    gs = source_embed @ Wfs                                     # [5,256]
    gc = b2 @ Wfp + bf[None, :]                                 # [9,256]
    G = np.stack([np.concatenate([gt, gs, gc[t : t + 1]], axis=0) for t in range(T)])

    # WG [50, T*256] band rows (DMA'd to SBUF partitions 0/32/64/96): for
    # expert t, col block [t*256, +128) holds the "half 0" stationary
    # operands, [t*256+128, +256) holds "half 1".
    WG = np.zeros((2 * KX + 2 * KU, T * D), dtype=f)
    for t in range(T):
        c = t * D
        WG[0:KX, c : c + 128] = W1e[t][:, 0:128]
        WG[KX : 2 * KX, c + 128 : c + 256] = W1e[t][:, 128:256]
        WG[2 * KX : 2 * KX + KU, c : c + 128] = G[t][:, 0:128]
        WG[2 * KX + KU : 2 * KX + 2 * KU, c + 128 : c + 256] = G[t][:, 128:256]

    bfl = ml_dtypes.bfloat16
    return (np.ascontiguousarray(WG.astype(bfl)),
            np.ascontiguousarray(VR.astype(bfl)))


def _runs_of(group_expert, b):
    """runs of equal expert inside block b: list of (t, c0, c1) col ranges."""
    GP = BLOCK // GRP
    g0 = b * GP
    experts = [int(group_expert[g0 + i]) for i in range(GP)]
    runs = []
    for i, t in enumerate(experts):
        if runs and runs[-1][0] == t:
            runs[-1] = (t, runs[-1][1], (i + 1) * GRP)
        else:
            runs.append((t, i * GRP, (i + 1) * GRP))
    return runs


def _build_program(m_t: tuple, L: int):
    """One compiled SPMD program for the given segment layout."""
    key = (m_t, L, _WARM_BURST, _XU_CHUNKS)
    if key in _PROGRAM_CACHE:
        return _PROGRAM_CACHE[key]

    group_expert = _group_experts(np.asarray(m_t, dtype=np.int64))
    NB = L // BLOCK

    nc = bacc.Bacc("TRN2", target_bir_lowering=False, debug=False,
                   num_devices=NCORES)
    xu_d = nc.dram_tensor("xu", [2 * KX + 2 * KU, L], _BF16, kind="ExternalInput")
    wg_d = nc.dram_tensor("wg", [2 * KX + 2 * KU, T * D], _BF16,
                          kind="ExternalInput")
    vr_d = nc.dram_tensor("vr", [128, T * 2 * D], _BF16, kind="ExternalInput")
    out_d = nc.dram_tensor("out", [128, 2 * L], _FP16, kind="ExternalOutput")

    RELU = mybir.ActivationFunctionType.Relu

    with tile.TileContext(nc) as tc:
        with (
            tc.tile_pool(name="wts", bufs=1) as wts,
            tc.tile_pool(name="hsb", bufs=3) as hsbp,
            tc.tile_pool(name="osb", bufs=3) as osbp,
            tc.tile_pool(name="hps", bufs=2, space=bass.MemorySpace.PSUM) as hps,
            tc.tile_pool(name="ops", bufs=2, space=bass.MemorySpace.PSUM) as ops,
        ):
            wg = wts.tile([128, T * D], _BF16)
            vr = wts.tile([128, T * 2 * D], _BF16)
            xu = wts.tile([128, L], _BF16)

            # scratch for the dependency-free warm burst; memsets go FIRST
            # on the vector queue so the PE can start at ~0.25us
            if _WARM_BURST:
                wmw = wts.tile([128, 128], _BF16)
                wma = wts.tile([128, 512], _BF16)
                nc.vector.memset(wmw[:], 0.0)
                nc.vector.memset(wma[:], 0.0)

            # Input DMA schedule, ordered by deadline. Per-partition
            # contiguous runs become single DMA packets, so keep them 4-9KB:
            # wg as 4 band rows, xu in quarter-L chunks. The first two
            # quarters ride the fast HWDGE queues (scalar + sync); the last
            # two (not needed until ~35us in) go to the idle gpsimd SW-DGE.
            bands = [(0, KX, 0), (KX, 2 * KX, 32),
                     (2 * KX, 2 * KX + KU, 64), (2 * KX + KU, 2 * KX + 2 * KU, 96)]
            nq = 4
            cw = math.ceil(L / nq / BLOCK) * BLOCK
            chunks = [(ci * cw, min((ci + 1) * cw, L)) for ci in range(nq)]
            chunks = [(c0, c1) for (c0, c1) in chunks if c0 < c1]

            def emit_xu_chunk(ci):
                c0, c1 = chunks[ci]
                for (r0, r1, p0) in bands:
                    nc.scalar.dma_start(xu[p0 : p0 + (r1 - r0), c0:c1],
                                        xu_d.ap()[r0:r1, c0:c1])

            # sync: wg bands, then vr (head split: first-used expert's slice
            # lands ~6us earlier so V(0) can start), then stores later.
            # scalar: only chunk 0 of xu up-front; later chunks are emitted
            # inside the block loop so their issue cost (~0.7us each) never
            # sits between the ACT relus and never delays relu(0).
            for (r0, r1, p0) in bands:
                nc.sync.dma_start(wg[p0 : p0 + (r1 - r0), :],
                                  wg_d.ap()[r0:r1, :])
            t0 = int(group_expert[0])
            v0, v1 = t0 * 2 * D, (t0 + 1) * 2 * D
            nc.sync.dma_start(vr[:, v0:v1], vr_d.ap()[:, v0:v1])
            if v0 > 0:
                nc.sync.dma_start(vr[:, 0:v0], vr_d.ap()[:, 0:v0])
            if v1 < T * 2 * D:
                nc.sync.dma_start(vr[:, v1:], vr_d.ap()[:, v1:])
            emit_xu_chunk(0)

            # dependency-free bf16 warm burst: keeps the PE busy (and the HAM
            # clock-gate open) until the vr weights land (~16us in). First 16
            # run cold (N=256), the rest warm (N=512 slices of the same tile).
            if _WARM_BURST:
                wmp = hps.tile([128, 2 * BLOCK], _F32, name="warmps", tag="hts")
                for i in range(_WARM_BURST):
                    n = 256
                    nc.tensor.matmul(wmp[:, 0:n], wmw[:], wma[:, 0:n],
                                     start=True, stop=True)

            hts = [None] * NB
            hsb = [None] * NB

            def emit_l1(b):
                """layer-1 pair: two concurrent row-group matmuls (rg 0/1)."""
                hts[b] = hps.tile([128, 2 * BLOCK], _F32, name=f"hts{b}",
                                  tag="hts")
                off = b * BLOCK
                for (t, c0, c1) in _runs_of(group_expert, b):
                    c = t * D
                    nc.tensor.matmul(
                        hts[b][:, c0:c1], wg[0:KX, c : c + 128],
                        xu[0:KX, off + c0 : off + c1], start=True, stop=True,
                        tile_position=(0, 0))
                    nc.tensor.matmul(
                        hts[b][:, BLOCK + c0 : BLOCK + c1],
                        wg[32 : 32 + KX, c + 128 : c + 256],
                        xu[32 : 32 + KX, off + c0 : off + c1],
                        start=True, stop=True, tile_position=(32, 0))

            def emit_hrelu(b):
                hsb[b] = hsbp.tile([128, 2 * BLOCK], _BF16, name=f"hsb{b}",
                                   tag="hsb")
                nc.scalar.activation(hsb[b][:], hts[b][:], RELU)

            for b in range(min(2, NB)):
                emit_l1(b)
                emit_hrelu(b)

            for b in range(NB):
                runs = _runs_of(group_expert, b)
                # L1(b+2) first: it merges into the same row-group session as
                # the previous block's G pair and gives the ACT relu a full
                # extra block of slack, so V(b+2) never waits on the relu.
                if b + 2 < NB:
                    emit_l1(b + 2)
                ots = ops.tile([128, 2 * BLOCK], _F32, name=f"ots{b}", tag="ots")
                # ---- all V matmuls (K=128, alternating PSUM banks), then
                # all G matmuls in ONE row-group session. start=True ONLY on
                # the block's first V pair: start clears has_written
                # bank-wide, and start=False on unwritten elements OVERWRITES
                # (proven: that is exactly how the earlier per-run-group bug
                # manifested), so later runs' first writes are safe with
                # start=False. This keeps multi-run blocks at one full-array
                # -> row-group mode switch round trip instead of one per run.
                off = b * BLOCK
                for ri, (t, c0, c1) in enumerate(runs):
                    c = t * D
                    for h in range(2):
                        for g in range(2):
                            nc.tensor.matmul(
                                ots[:, g * BLOCK + c0 : g * BLOCK + c1],
                                vr[:, (t * 2 + h) * D + g * 128
                                   : (t * 2 + h) * D + (g + 1) * 128],
                                hsb[b][:, h * BLOCK + c0 : h * BLOCK + c1],
                                start=(h == 0 and ri == 0), stop=False,
                                skip_group_check=True)
                for ri, (t, c0, c1) in enumerate(runs):
                    c = t * D
                    last = ri == len(runs) - 1
                    nc.tensor.matmul(
                        ots[:, c0:c1], wg[64 : 64 + KU, c : c + 128],
                        xu[64 : 64 + KU, off + c0 : off + c1],
                        start=False, stop=last, tile_position=(64, 0),
                        skip_group_check=True)
                    nc.tensor.matmul(
                        ots[:, BLOCK + c0 : BLOCK + c1],
                        wg[96 : 96 + KU, c + 128 : c + 256],
                        xu[96 : 96 + KU, off + c0 : off + c1],
                        start=False, stop=last, tile_position=(96, 0),
                        skip_group_check=True)
                # later xu chunks: issue from the scalar queue between relus,
                # well ahead of each chunk's first-use deadline
                if b in (1, 4, 8):
                    emit_xu_chunk(b // 3 + 1)
                # ---- evacuate: out relu -> fp16 on DVE, h relu on ACT.
                # Stores batch 4 blocks into one staging tile so each store
                # has 8KB contiguous per-partition runs (one DMA packet).
                if b % SBATCH == 0:
                    osb = osbp.tile([128, SBATCH * 2 * BLOCK], _FP16,
                                    name=f"osb{b // SBATCH}", tag="osb")
                s = (b % SBATCH) * 2 * BLOCK
                nc.vector.tensor_scalar_max(osb[:, s : s + 2 * BLOCK], ots[:],
                                            0.0)
                if b + 2 < NB:
                    emit_hrelu(b + 2)
                if b % SBATCH == SBATCH - 1 or b == NB - 1:
                    b0 = (b // SBATCH) * SBATCH
                    # stores stay on sync: a dma_start occupies the issuing
                    # engine ~0.7us, which would congest the ACT relu stream
                    nc.sync.dma_start(
                        out_d.ap()[:, b0 * 2 * BLOCK : (b + 1) * 2 * BLOCK],
                        osb[:, 0 : (b + 1 - b0) * 2 * BLOCK])

    nc.compile()
    _PROGRAM_CACHE[key] = nc
    return nc


def kernel(type_ids, source_ids, params, type_embed, source_embed,
           W1, b1, W2, b2, Wf, bf):
    global LAST_RESULT
    type_ids = np.asarray(type_ids, dtype=np.int32)
    source_ids = np.asarray(source_ids, dtype=np.int32)
    params = np.asarray(params, dtype=np.float32)
    E = type_ids.shape[0]

    base_ids = BASE_MAP[type_ids]
    n_t, m_t, L = _layout(base_ids)
    ORD = _build_order(base_ids, n_t, m_t, L)
    XU = _host_inputs(type_ids, source_ids, params, ORD)
    WG, VR = _host_weights(
        np.asarray(type_embed), np.asarray(source_embed),
        np.asarray(W1), np.asarray(b1), np.asarray(W2), np.asarray(b2),
        np.asarray(Wf), np.asarray(bf))

    nc = _build_program(tuple(int(v) for v in m_t), L)

    in_maps = [{"xu": XU[c], "wg": WG, "vr": VR} for c in range(NCORES)]

    trace = bool(int(os.environ.get("EDGEENC_TRACE", "0")))
    res = run_bass_kernel_spmd(nc, in_maps, core_ids=list(range(NCORES)),
                               trace=trace)
    LAST_RESULT = res

    NB = L // BLOCK
    full = np.zeros((E, D), dtype=np.float32)
    for c in range(NCORES):
        sel = ORD[c] >= 0
        oc = np.asarray(res.results[c]["out"])         # [128, 2L] fp16
        oc = oc.reshape(128, NB, 2, BLOCK).transpose(2, 0, 1, 3).reshape(D, L)
        full[ORD[c][sel]] = oc[:, sel].T.astype(np.float32)
    return full
